# revision 38
# baseline (speedup 1.0000x reference)
"""Trainium2 Bass kernel for nn_Attention_40475771798025.

Full attention layer: QKV projection + RoPE + GQA causal attention + output
projection. B=2, S=2048, D=4096, H=32 q-heads, KV=8 kv-heads, HD=128.

Sharding: head-parallel tensor parallelism across 8 cores. Core g owns kv-head
g (its 4 q-heads, 1 k-head, 1 v-head) for both batches. Weights are
pre-transposed on the host; x/w/wo stream in bf16 (halves HBM traffic, same
PE rate as fp32r), QKV accumulate in fp32 SBUF. The output projection
produces per-core partial sums of the full [T, D] output in bf16, summed on
the host.

Device kernel per core, per batch:
  A: qkv^T = wqkvT^T @ x^T accumulated over graded D-chunk passes (SBUF f32
     accumulation, paired-tile PSUM evacuation). RoPE (rotation-matrix matmul
     + DVE/GPSIMD combine) folded into the last pass per m-slot; V transposed
     to natural bf16 layout with PE transposes. The first attention block's
     scores+exp are pre-issued at the tail of A (between the remaining q-head
     passes) so the EV pipeline starts full.
  B+C fused, q-tile outer / head inner, qt order [1,0,2,3] (qt0's tiny
     4-k-tile blocks run when C work and a deep pipeline can cover their
     latency): scores^T tiles [128k, <=512q] on PE with causal column
     restriction (straddle tiles only compute unmasked column ranges; f32r
     matmuls keep N>=256, bf16 EV/denominator use exact offsets), additive
     -1e9 mask on the diagonal blocks (DVE), exp on ACT -> bf16 e tiles,
     E@V + all-ones denominator accumulate in PSUM through a global
     depth-PIPE software pipeline crossing q-tile/head boundaries (no drain
     stalls). Normalized bf16 attention outputs feed the output projection:
     each finished q-tile's token-tiles of out = attT^T @ woT are emitted at
     the starts of following attention blocks, filling exp-latency gaps on
     PE and spreading the output DMA.

Loads ride the SP hardware DMA queue, stores the ACT queue, so next-batch x
prefetch (x/w pools live at top level) is not stuck behind output writes.
"""
import sys
sys.path.insert(0, "/opt/trn_rl_repo")
import numpy as np

B, S, D = 2, 2048, 4096
H, KV, HD = 32, 8, 128
REP = H // KV            # 4 q-heads per core
T = B * S                # 4096 flattened tokens
NCORES = 8
P = 128
QTW, KTW = 512, 128      # q-tile width (psum free dim), k-tile width
MQKV = REP + 2           # 6 m-tiles of 128: q0..q3, k, v
KIDX, VIDX = REP, REP + 1
SCALE = 1.0 / float(np.sqrt(HD))
NT = S // QTW            # 4 q-tiles per batch
NKT = S // KTW           # 16 k-tiles per batch
TPQ = QTW // P           # 4 token tiles per q-tile
JT = QTW // KTW          # 4 straddle positions
CSIZES = [2, 3, 4, 4, 4, 4, 4, 4, 3]   # D-contraction pass sizes (128-chunks)
PIPE = 12                # EV pipeline depth (e tiles in flight)
QT_ORDER = [1, 0, 2, 3]
OFFS = {0: 0, 1: 128, 2: 256, 3: 256}  # scores col offset (f32r needs N>=256)
OFFE = {0: 0, 1: 128, 2: 256, 3: 384}  # EV/denominator col offset (bf16)

_nc = None


def _build_nc(reps=1):
    import concourse.bacc as bacc
    import concourse.mybir as mybir
    import concourse.tile as tile
    from contextlib import ExitStack

    F32 = mybir.dt.float32
    F32R = mybir.dt.float32r
    BF16 = mybir.dt.bfloat16

    nc = bacc.Bacc("TRN2")
    xT_d = nc.dram_tensor("xT", (D, T), BF16, kind="ExternalInput")
    wqkvT_d = nc.dram_tensor("wqkvT", (D, MQKV * P), BF16,
                             kind="ExternalInput")
    woT_d = nc.dram_tensor("woT", (REP * P, D), BF16, kind="ExternalInput")
    cdup_d = nc.dram_tensor("cdup", (P, T), F32, kind="ExternalInput")
    sdup_d = nc.dram_tensor("sdup", (P, T), F32, kind="ExternalInput")
    pt_d = nc.dram_tensor("pt", (P, P), F32, kind="ExternalInput")
    ones_d = nc.dram_tensor("ones", (P, P), BF16, kind="ExternalInput")
    ident_d = nc.dram_tensor("ident", (P, P), F32, kind="ExternalInput")
    mtri_d = nc.dram_tensor("mtri", (P, 2 * KTW), F32, kind="ExternalInput")
    out_d = nc.dram_tensor("out", (T, D), BF16, kind="ExternalOutput")

    NPASS = len(CSIZES)
    assert sum(CSIZES) == D // P

    with tile.TileContext(nc) as tc, ExitStack() as top:
        persist = top.enter_context(tc.tile_pool(name="persist", bufs=1))
        wop = top.enter_context(tc.tile_pool(name="wo", bufs=1))
        accp = top.enter_context(tc.tile_pool(name="acc", bufs=1))
        vnp = top.enter_context(tc.tile_pool(name="vnat", bufs=1))
        csp = top.enter_context(tc.tile_pool(name="cs", bufs=1))
        xqp = top.enter_context(tc.tile_pool(name="xq", bufs=3))
        wqp = top.enter_context(tc.tile_pool(name="wql", bufs=2))
        tmpp = top.enter_context(tc.tile_pool(name="tmp", bufs=2))
        ep = top.enter_context(tc.tile_pool(name="e", bufs=PIPE + 1))
        rp = top.enter_context(tc.tile_pool(name="rec", bufs=2))
        attp = top.enter_context(tc.tile_pool(name="att", bufs=2))
        obp = top.enter_context(tc.tile_pool(name="ob", bufs=2))

        pt_s = persist.tile([P, P], F32R)
        ones_s = persist.tile([P, P], BF16)
        ident_s = persist.tile([P, P], F32R)
        mtri_s = persist.tile([P, 2 * KTW], F32)
        nc.scalar.dma_start(pt_s[:], pt_d[:].bitcast(F32R))
        nc.scalar.dma_start(ones_s[:], ones_d[:])
        nc.scalar.dma_start(ident_s[:], ident_d[:].bitcast(F32R))
        nc.scalar.dma_start(mtri_s[:], mtri_d[:])
        # wo/cos/sin loads are deferred into mid-phase-A where the DMA queue
        # has slack, so they don't delay the first projection passes.
        wo_s = wop.tile([P, REP, D], BF16)
        wo_loaded = [False]

        def load_wo():
            if not wo_loaded[0]:
                wo_loaded[0] = True
                nc.scalar.dma_start(
                    wo_s[:], woT_d[:].rearrange("(h p) n -> p h n", p=P))

        # acc: [128, m, S]; m = q0..q3, k, v. fp32(r) so matmuls can consume
        # it; rope overwrites slots in place. Reused across batches.
        acc = accp.tile([P, MQKV, S], F32R)
        v_nat = vnp.tile([P, NKT, HD], BF16)

        for _rep in range(reps):
          for b in range(B):
            bsl = slice(b * S, (b + 1) * S)

            cdup_b = csp.tile([P, NT, QTW], F32, tag="c")
            sdup_b = csp.tile([P, NT, QTW], F32, tag="s")

            def load_cs(cdup_b=cdup_b, sdup_b=sdup_b, bsl=bsl):
                nc.scalar.dma_start(
                    cdup_b[:],
                    cdup_d[:, bsl].rearrange("p (n q) -> p n q", q=QTW))
                nc.scalar.dma_start(
                    sdup_b[:],
                    sdup_d[:, bsl].rearrange("p (n q) -> p n q", q=QTW))

            def rope_m(m, psRot, cdup_b=cdup_b, sdup_b=sdup_b):
                """RoPE in place on acc slot m (rotation matmul + combine)."""
                for tt in range(NT):
                    tsl = slice(tt * QTW, (tt + 1) * QTW)
                    accsl = acc[:, m, tsl]
                    rps = psRot.tile([P, QTW], F32, tag="rot")
                    nc.tensor.matmul(rps[:], lhsT=pt_s[:], rhs=accsl,
                                     start=True, stop=True)
                    t1 = tmpp.tile([P, QTW], F32, tag="t1")
                    t2 = tmpp.tile([P, QTW], F32, tag="t2")
                    nc.gpsimd.tensor_mul(t1[:], accsl.bitcast(F32),
                                         cdup_b[:, tt, :])
                    nc.vector.tensor_mul(t2[:], rps[:], sdup_b[:, tt, :])
                    nc.vector.tensor_add(accsl, t1[:], t2[:])

            # attention pipeline state (entries appended in A's tail and in
            # the fused B+C phase; all pops happen in the B+C phase)
            pend = []    # in-flight e tiles awaiting EV+denominator
            cqueue = []  # (att dict, qt, ttl) token-tiles ready for C
            att_first = {}

            def emit_scores(qt, h, cur_att, psget):
                """Scores + mask + exp for one (qt, h) block; appends pend."""
                nkt = (qt + 1) * JT
                slot = {"po": None, "pd": None}
                for kt in range(nkt):
                    j = kt - qt * JT
                    off = OFFS[j] if j >= 0 else 0
                    eo = OFFE[j] if j >= 0 else 0
                    ps_s = psget()
                    nc.tensor.matmul(
                        ps_s[:, off:],
                        lhsT=acc[:, KIDX, kt * KTW:(kt + 1) * KTW],
                        rhs=acc[:, h, qt * QTW + off:(qt + 1) * QTW],
                        start=True, stop=True)
                    if j >= 0:
                        if j < JT - 1:
                            # diagonal 128-block triangle mask
                            nc.vector.tensor_add(
                                ps_s[:, j * KTW:(j + 1) * KTW],
                                ps_s[:, j * KTW:(j + 1) * KTW],
                                mtri_s[:, KTW:])
                        else:
                            # [256:384) fully masked + [384:) triangle
                            nc.vector.tensor_add(
                                ps_s[:, 2 * KTW:], ps_s[:, 2 * KTW:],
                                mtri_s[:])
                    e = ep.tile([P, QTW], BF16, tag="e")
                    nc.scalar.activation(
                        e[:, off:], ps_s[:, off:],
                        mybir.ActivationFunctionType.Exp, scale=SCALE)
                    pend.append(dict(
                        e=e, kt=kt, nkt=nkt, slot=slot, h=h, qt=qt,
                        att=cur_att, eo=eo))
                    yield

            # ---- phase A: projections + rope + v transpose ----
            with ExitStack() as actx:
                psA = actx.enter_context(
                    tc.tile_pool(name="psA", bufs=2, space="PSUM"))
                psVT = actx.enter_context(
                    tc.tile_pool(name="psVT", bufs=1, space="PSUM"))
                psRot = actx.enter_context(
                    tc.tile_pool(name="psRot", bufs=1, space="PSUM"))

                c_off = 0
                for dq, c_n in enumerate(CSIZES):
                    dsl = slice(c_off * P, (c_off + c_n) * P)
                    c_off += c_n
                    xq = xqp.tile([P, c_n, S], BF16, tag="xq",
                                  padded_shape=[P, max(CSIZES), S])
                    xsrc = xT_d[dsl, bsl].rearrange("(c p) t -> p c t", p=P)
                    wql = wqp.tile([P, c_n, MQKV * P], BF16, tag="wql",
                                   padded_shape=[P, max(CSIZES), MQKV * P])
                    wsrc = wqkvT_d[dsl, :].rearrange("(c p) m -> p c m", p=P)
                    if dq == 0:
                        # k/v weights + first x chunk land first so the PE
                        # starts as early as possible
                        nc.sync.dma_start(wql[:, :, KIDX * P:],
                                          wsrc[:, :, KIDX * P:])
                        nc.sync.dma_start(xq[:, :, :QTW], xsrc[:, :, :QTW])
                        nc.sync.dma_start(wql[:, :, :KIDX * P],
                                          wsrc[:, :, :KIDX * P])
                        for tt in range(1, NT):
                            tsl = slice(tt * QTW, (tt + 1) * QTW)
                            nc.sync.dma_start(xq[:, :, tsl], xsrc[:, :, tsl])
                    else:
                        nc.sync.dma_start(wql[:], wsrc[:])
                        nc.sync.dma_start(xq[:], xsrc[:])
                    if dq == 3:
                        load_cs()
                    elif dq == 4:
                        load_wo()
                    last = dq == NPASS - 1
                    for m in [KIDX, 0, VIDX, 1, 2, 3]:
                        for tp in range(NT // 2):   # tt pairs share a psum
                            ps = psA.tile([P, 2 * QTW], F32, tag="pa")
                            for c in range(c_n):
                                for half in range(2):
                                    tt = tp * 2 + half
                                    nc.tensor.matmul(
                                        ps[:, half * QTW:(half + 1) * QTW],
                                        lhsT=wql[:, c, m * P:(m + 1) * P],
                                        rhs=xq[:, c,
                                               tt * QTW:(tt + 1) * QTW],
                                        start=(c == 0), stop=(c == c_n - 1))
                            accsl = acc[:, m,
                                        tp * 2 * QTW:(tp + 1) * 2 * QTW]
                            if dq == 0:
                                nc.scalar.copy(accsl, ps[:])
                            else:
                                nc.vector.tensor_add(
                                    accsl, accsl.bitcast(F32), ps[:])
                        if last:
                            if m == VIDX:
                                for c in range(NKT):  # v -> natural layout
                                    tps = psVT.tile([P, 2, P], F32R,
                                                    tag="vt")
                                    nc.tensor.transpose(
                                        tps[:, c % 2, :],
                                        acc[:, m, c * P:(c + 1) * P],
                                        ident_s[:])
                                    if c % 2 == 0:
                                        nc.scalar.copy(v_nat[:, c, :],
                                                       tps[:, 0, :])
                                    else:
                                        nc.vector.tensor_copy(
                                            v_nat[:, c, :], tps[:, 1, :])
                                # pre-issue first attention block: its exps
                                # overlap the remaining q-head passes/ropes
                                for _ in emit_scores(
                                        QT_ORDER[0], 0, att_first,
                                        lambda: psA.tile([P, QTW], F32, tag="s0",
                                                         name="ps_s0")):
                                    pass
                            else:
                                rope_m(m, psRot)

            # ---- fused B+C: attention + output projection ----
            with ExitStack() as bctx:
                psS = bctx.enter_context(
                    tc.tile_pool(name="psS", bufs=4, space="PSUM"))
                psO = bctx.enter_context(
                    tc.tile_pool(name="psO", bufs=2, space="PSUM"))
                psD = bctx.enter_context(
                    tc.tile_pool(name="psD", bufs=2, space="PSUM"))

                def emit_C(att, qt, ttl, b=b):
                    tt0 = qt * TPQ + ttl
                    for ho in range(2):
                        ob = obp.tile([P, D // 2], BF16, tag="ob")
                        for hf in range(D // QTW // 2):
                            half = ho * (D // QTW // 2) + hf
                            ps = psS.tile([P, QTW], F32, tag="s")
                            for hh in range(REP):
                                nc.tensor.matmul(
                                    ps[:],
                                    lhsT=att[hh][:, ttl * P:(ttl + 1) * P],
                                    rhs=wo_s[:, hh,
                                             half * QTW:(half + 1) * QTW],
                                    start=(hh == 0), stop=(hh == REP - 1))
                            if half % 2 == 0:
                                nc.scalar.copy(
                                    ob[:, hf * QTW:(hf + 1) * QTW], ps[:])
                            else:
                                nc.vector.tensor_copy(
                                    ob[:, hf * QTW:(hf + 1) * QTW], ps[:])
                        nc.scalar.dma_start(
                            out_d[b * S + tt0 * P:b * S + (tt0 + 1) * P,
                                  ho * (D // 2):(ho + 1) * (D // 2)],
                            ob[:])

                def pop_one():
                    en = pend.pop(0)
                    slot = en["slot"]
                    if slot["po"] is None:
                        slot["po"] = psO.tile([P, QTW], F32, tag="o", name="ps_o")
                        slot["pd"] = psD.tile([P, QTW], F32, tag="d", name="ps_d")
                    eo = en["eo"]
                    first = en["kt"] == 0
                    final = en["kt"] == en["nkt"] - 1
                    nc.tensor.matmul(
                        slot["po"][:, eo:], lhsT=v_nat[:, en["kt"], :],
                        rhs=en["e"][:, eo:], start=first, stop=final,
                        skip_group_check=True)
                    nc.tensor.matmul(
                        slot["pd"][:, eo:], lhsT=ones_s[:],
                        rhs=en["e"][:, eo:], start=first, stop=final,
                        skip_group_check=True)
                    if final:
                        rec = rp.tile([P, QTW], F32, tag="rec")
                        nc.vector.reciprocal(rec[:], slot["pd"][:])
                        at = attp.tile([P, QTW], BF16, tag=f"at{en['h']}")
                        nc.vector.tensor_mul(at[:], slot["po"][:], rec[:])
                        en["att"][en["h"]] = at
                        if en["h"] == REP - 1:
                            for ttl in range(TPQ):
                                cqueue.append((en["att"], en["qt"], ttl))

                for qi, qt in enumerate(QT_ORDER):
                    cur_att = att_first if qi == 0 else {}
                    for h in range(REP):
                        if qi == 0 and h == 0:
                            continue    # pre-issued in phase A's tail
                        if cqueue:
                            emit_C(*cqueue.pop(0))
                        if len(cqueue) > 3:
                            emit_C(*cqueue.pop(0))
                        for _ in emit_scores(
                                qt, h, cur_att,
                                lambda: psS.tile([P, QTW], F32, tag="s",
                                                 name="ps_sc")):
                            while len(pend) > PIPE:
                                pop_one()
                        if h == REP - 1:
                            # drain harder so this qt's C work unlocks
                            # before the next (possibly tiny) blocks start
                            while len(pend) > PIPE // 2:
                                pop_one()
                while pend:
                    pop_one()
                while cqueue:
                    emit_C(*cqueue.pop(0))
    nc.compile()
    return nc


def get_nc():
    global _nc
    if _nc is None:
        _nc = _build_nc()
    return _nc


def make_in_maps(x, freqs_cos, freqs_sin, wq, wk, wv, wo):
    """Host-side prep: transposes, rope tables, masks, per-core weight shards."""
    import concourse.mybir as mybir
    BF16 = mybir.dt.np(mybir.dt.bfloat16)
    x = np.ascontiguousarray(x, np.float32)
    fc = np.asarray(freqs_cos, np.float32)
    fs = np.asarray(freqs_sin, np.float32)
    wq = np.asarray(wq, np.float32)
    wk = np.asarray(wk, np.float32)
    wv = np.asarray(wv, np.float32)
    wo = np.asarray(wo, np.float32)

    xT = np.ascontiguousarray(x.reshape(T, D).astype(BF16).T)
    cdup = np.ascontiguousarray(np.tile(np.repeat(fc.T, 2, axis=0), (1, B)))
    sdup = np.ascontiguousarray(np.tile(np.repeat(fs.T, 2, axis=0), (1, B)))
    prot = np.zeros((P, P), np.float32)
    for i in range(P // 2):
        prot[2 * i, 2 * i + 1] = -1.0
        prot[2 * i + 1, 2 * i] = 1.0
    pt = np.ascontiguousarray(prot.T)
    ones = np.ones((P, P), BF16)
    ident = np.eye(P, dtype=np.float32)
    ki = np.arange(KTW)[:, None]
    ci = np.arange(KTW)[None, :]
    tri = np.where(ki > ci, -1e9, 0.0).astype(np.float32)
    mtri = np.concatenate([np.full((P, KTW), -1e9, np.float32), tri], axis=1)
    mtri = np.ascontiguousarray(mtri)

    in_maps = []
    for g in range(NCORES):
        wq_g = wq[g * REP * HD:(g + 1) * REP * HD]
        wk_g = wk[g * HD:(g + 1) * HD]
        wv_g = wv[g * HD:(g + 1) * HD]
        wqkvT = np.ascontiguousarray(
            np.concatenate([wq_g, wk_g, wv_g], 0).astype(BF16).T)
        woT = np.ascontiguousarray(
            wo[:, g * REP * HD:(g + 1) * REP * HD].T.astype(BF16))
        in_maps.append({
            "xT": xT, "wqkvT": wqkvT, "woT": woT,
            "cdup": cdup, "sdup": sdup, "pt": pt, "ones": ones,
            "ident": ident, "mtri": mtri,
        })
    return in_maps


def kernel(x, freqs_cos, freqs_sin, wq, wk, wv, wo):
    from concourse.bass_utils import run_bass_kernel_spmd
    nc = get_nc()
    in_maps = make_in_maps(x, freqs_cos, freqs_sin, wq, wk, wv, wo)
    res = run_bass_kernel_spmd(nc, in_maps, core_ids=list(range(NCORES)))
    out = np.zeros((T, D), np.float64)
    for r in res.results:
        out += r["out"].astype(np.float64)
    return out.astype(np.float32).reshape(B, S, D)


# revision 42
# speedup vs baseline: 1.4443x; 1.4443x over previous
"""Trainium2 Bass kernel for nn_Attention_40475771798025.

Full attention layer: QKV projection + RoPE + GQA causal attention + output
projection. B=2, S=2048, D=4096, H=32 q-heads, KV=8 kv-heads, HD=128.

Sharding: head-parallel tensor parallelism across 8 cores. Core g owns kv-head
g (its 4 q-heads, 1 k-head, 1 v-head) for both batches. Weights are
pre-transposed on the host; x/w/wo stream in bf16 (halves HBM traffic, same
PE rate as fp32r), QKV accumulate in fp32 SBUF. The output projection
produces per-core partial sums of the full [T, D] output in bf16, summed on
the host.

Device kernel per core, per batch:
  A: qkv^T = wqkvT^T @ x^T accumulated over graded D-chunk passes (SBUF f32
     accumulation, paired-tile PSUM evacuation). RoPE (rotation-matrix matmul
     + DVE/GPSIMD combine) folded into the last pass per m-slot; V transposed
     to natural bf16 layout with PE transposes. The first attention block's
     scores+exp are pre-issued at the tail of A (between the remaining q-head
     passes) so the EV pipeline starts full.
  B+C fused, q-tile outer / head inner, qt order [1,0,2,3] (qt0's tiny
     4-k-tile blocks run when C work and a deep pipeline can cover their
     latency): scores^T tiles [128k, <=512q] on PE with causal column
     restriction (straddle tiles only compute unmasked column ranges; f32r
     matmuls keep N>=256, bf16 EV/denominator use exact offsets), additive
     -1e9 mask on the diagonal blocks (DVE), exp on ACT -> bf16 e tiles,
     E@V + all-ones denominator accumulate in PSUM through a global
     depth-PIPE software pipeline crossing q-tile/head boundaries (no drain
     stalls). Normalized bf16 attention outputs feed the output projection:
     each finished q-tile's token-tiles of out = attT^T @ woT are emitted at
     the starts of following attention blocks, filling exp-latency gaps on
     PE and spreading the output DMA.

Loads ride the SP hardware DMA queue, stores the ACT queue, so next-batch x
prefetch (x/w pools live at top level) is not stuck behind output writes.
"""
import sys
sys.path.insert(0, "/opt/trn_rl_repo")
import numpy as np

B, S, D = 2, 2048, 4096
H, KV, HD = 32, 8, 128
REP = H // KV            # 4 q-heads per core
T = B * S                # 4096 flattened tokens
NCORES = 8
P = 128
QTW, KTW = 512, 128      # q-tile width (psum free dim), k-tile width
MQKV = REP + 2           # 6 m-tiles of 128: q0..q3, k, v
KIDX, VIDX = REP, REP + 1
SCALE = 1.0 / float(np.sqrt(HD))
NT = S // QTW            # 4 q-tiles per batch
NKT = S // KTW           # 16 k-tiles per batch
TPQ = QTW // P           # 4 token tiles per q-tile
JT = QTW // KTW          # 4 straddle positions
CSIZES = [2, 3, 4, 4, 4, 4, 4, 4, 3]   # D-contraction pass sizes (128-chunks)
PIPE = 10                # EV pipeline depth (e tiles in flight)
QT_ORDER = [1, 0, 2, 3]
OFFS = {0: 0, 1: 128, 2: 256, 3: 256}  # scores col offset (f32r needs N>=256)
OFFE = {0: 0, 1: 128, 2: 256, 3: 384}  # EV/denominator col offset (bf16)

_nc = None


def _build_nc(reps=1):
    import concourse.bacc as bacc
    import concourse.mybir as mybir
    import concourse.tile as tile
    from contextlib import ExitStack

    F32 = mybir.dt.float32
    F32R = mybir.dt.float32r
    BF16 = mybir.dt.bfloat16

    nc = bacc.Bacc("TRN2")
    xT_d = nc.dram_tensor("xT", (D, T), BF16, kind="ExternalInput")
    wqkvT_d = nc.dram_tensor("wqkvT", (D, MQKV * P), BF16,
                             kind="ExternalInput")
    woT_d = nc.dram_tensor("woT", (REP * P, D), BF16, kind="ExternalInput")
    cdup_d = nc.dram_tensor("cdup", (P, T), F32, kind="ExternalInput")
    sdup_d = nc.dram_tensor("sdup", (P, T), F32, kind="ExternalInput")
    pt_d = nc.dram_tensor("pt", (P, P), F32, kind="ExternalInput")
    ones_d = nc.dram_tensor("ones", (P, P), BF16, kind="ExternalInput")
    ident_d = nc.dram_tensor("ident", (P, P), F32, kind="ExternalInput")
    mtri_d = nc.dram_tensor("mtri", (P, 2 * KTW), F32, kind="ExternalInput")
    out_d = nc.dram_tensor("out", (T, D), BF16, kind="ExternalOutput")

    NPASS = len(CSIZES)
    assert sum(CSIZES) == D // P

    with tile.TileContext(nc) as tc, ExitStack() as top:
        persist = top.enter_context(tc.tile_pool(name="persist", bufs=1))
        wop = top.enter_context(tc.tile_pool(name="wo", bufs=1))
        accp = top.enter_context(tc.tile_pool(name="acc", bufs=1))
        vnp = top.enter_context(tc.tile_pool(name="vnat", bufs=1))
        csp = top.enter_context(tc.tile_pool(name="cs", bufs=1))
        xqp = top.enter_context(tc.tile_pool(name="xq", bufs=3))
        wqp = top.enter_context(tc.tile_pool(name="wql", bufs=2))
        tmpp = top.enter_context(tc.tile_pool(name="tmp", bufs=2))
        ep = top.enter_context(tc.tile_pool(name="e", bufs=PIPE + 2))
        rp = top.enter_context(tc.tile_pool(name="rec", bufs=2))
        attp = top.enter_context(tc.tile_pool(name="att", bufs=2))
        obp = top.enter_context(tc.tile_pool(name="ob", bufs=2))

        pt_s = persist.tile([P, P], F32R)
        ones_s = persist.tile([P, P], BF16)
        ident_s = persist.tile([P, P], F32R)
        mtri_s = persist.tile([P, 2 * KTW], F32)
        nc.scalar.dma_start(pt_s[:], pt_d[:].bitcast(F32R))
        nc.scalar.dma_start(ones_s[:], ones_d[:])
        nc.scalar.dma_start(ident_s[:], ident_d[:].bitcast(F32R))
        nc.scalar.dma_start(mtri_s[:], mtri_d[:])
        # wo/cos/sin loads are deferred into mid-phase-A where the DMA queue
        # has slack, so they don't delay the first projection passes.
        wo_s = wop.tile([P, REP, D], BF16)
        wo_loaded = [False]

        def load_wo():
            if not wo_loaded[0]:
                wo_loaded[0] = True
                nc.scalar.dma_start(
                    wo_s[:], woT_d[:].rearrange("(h p) n -> p h n", p=P))

        # acc: [128, m, S]; m = q0..q3, k, v. fp32(r) so matmuls can consume
        # it; rope overwrites slots in place. Reused across batches.
        acc = accp.tile([P, MQKV, S], F32R)
        v_nat = vnp.tile([P, NKT, HD], BF16)

        for _rep in range(reps):
          for b in range(B):
            bsl = slice(b * S, (b + 1) * S)

            cdup_b = csp.tile([P, NT, QTW], F32, tag="c")
            sdup_b = csp.tile([P, NT, QTW], F32, tag="s")

            def load_cs(cdup_b=cdup_b, sdup_b=sdup_b, bsl=bsl):
                nc.scalar.dma_start(
                    cdup_b[:],
                    cdup_d[:, bsl].rearrange("p (n q) -> p n q", q=QTW))
                nc.scalar.dma_start(
                    sdup_b[:],
                    sdup_d[:, bsl].rearrange("p (n q) -> p n q", q=QTW))

            def rope_m(m, psRot, cdup_b=cdup_b, sdup_b=sdup_b):
                """RoPE in place on acc slot m (rotation matmul + combine)."""
                for tt in range(NT):
                    tsl = slice(tt * QTW, (tt + 1) * QTW)
                    accsl = acc[:, m, tsl]
                    rps = psRot.tile([P, QTW], F32, tag="rot")
                    nc.tensor.matmul(rps[:], lhsT=pt_s[:], rhs=accsl,
                                     start=True, stop=True)
                    t1 = tmpp.tile([P, QTW], F32, tag="t1")
                    t2 = tmpp.tile([P, QTW], F32, tag="t2")
                    nc.gpsimd.tensor_mul(t1[:], accsl.bitcast(F32),
                                         cdup_b[:, tt, :])
                    nc.vector.tensor_mul(t2[:], rps[:], sdup_b[:, tt, :])
                    nc.vector.tensor_add(accsl, t1[:], t2[:])

            # attention pipeline state (entries appended in A's tail and in
            # the fused B+C phase; all pops happen in the B+C phase)
            pend = []    # in-flight e tiles awaiting EV+denominator
            cqueue = []  # (att dict, qt, ttl) token-tiles ready for C
            att_first = {}

            def emit_scores(qt, h, cur_att, psget):
                """Scores + mask + exp for one (qt, h) block; appends pend."""
                nkt = (qt + 1) * JT
                slot = {"po": None, "pd": None}
                for kt in range(nkt):
                    j = kt - qt * JT
                    off = OFFS[j] if j >= 0 else 0
                    eo = OFFE[j] if j >= 0 else 0
                    ps_s = psget()
                    nc.tensor.matmul(
                        ps_s[:, off:],
                        lhsT=acc[:, KIDX, kt * KTW:(kt + 1) * KTW],
                        rhs=acc[:, h, qt * QTW + off:(qt + 1) * QTW],
                        start=True, stop=True)
                    if j >= 0:
                        if j < JT - 1:
                            # diagonal 128-block triangle mask
                            nc.vector.tensor_add(
                                ps_s[:, j * KTW:(j + 1) * KTW],
                                ps_s[:, j * KTW:(j + 1) * KTW],
                                mtri_s[:, KTW:])
                        else:
                            # [256:384) fully masked + [384:) triangle
                            nc.vector.tensor_add(
                                ps_s[:, 2 * KTW:], ps_s[:, 2 * KTW:],
                                mtri_s[:])
                    e = ep.tile([P, QTW], BF16, tag="e")
                    nc.scalar.activation(
                        e[:, off:], ps_s[:, off:],
                        mybir.ActivationFunctionType.Exp, scale=SCALE)
                    pend.append(dict(
                        e=e, kt=kt, nkt=nkt, slot=slot, h=h, qt=qt,
                        att=cur_att, eo=eo))
                    yield

            # ---- phase A: projections + rope + v transpose ----
            with ExitStack() as actx:
                psA = actx.enter_context(
                    tc.tile_pool(name="psA", bufs=3, space="PSUM"))
                psVT = actx.enter_context(
                    tc.tile_pool(name="psVT", bufs=1, space="PSUM"))
                psRot = actx.enter_context(
                    tc.tile_pool(name="psRot", bufs=1, space="PSUM"))

                c_off = 0
                for dq, c_n in enumerate(CSIZES):
                    dsl = slice(c_off * P, (c_off + c_n) * P)
                    c_off += c_n
                    xq = xqp.tile([P, c_n, S], BF16, tag="xq",
                                  padded_shape=[P, max(CSIZES), S])
                    xsrc = xT_d[dsl, bsl].rearrange("(c p) t -> p c t", p=P)
                    wql = wqp.tile([P, c_n, MQKV * P], BF16, tag="wql",
                                   padded_shape=[P, max(CSIZES), MQKV * P])
                    wsrc = wqkvT_d[dsl, :].rearrange("(c p) m -> p c m", p=P)
                    if dq == 0:
                        # k/v weights + first x chunk land first so the PE
                        # starts as early as possible
                        nc.sync.dma_start(wql[:, :, KIDX * P:],
                                          wsrc[:, :, KIDX * P:])
                        nc.sync.dma_start(xq[:, :, :QTW], xsrc[:, :, :QTW])
                        nc.sync.dma_start(wql[:, :, :KIDX * P],
                                          wsrc[:, :, :KIDX * P])
                        for tt in range(1, NT):
                            tsl = slice(tt * QTW, (tt + 1) * QTW)
                            nc.sync.dma_start(xq[:, :, tsl], xsrc[:, :, tsl])
                    else:
                        nc.sync.dma_start(wql[:], wsrc[:])
                        nc.sync.dma_start(xq[:], xsrc[:])
                    if dq == 3:
                        load_cs()
                    elif dq == 4:
                        load_wo()
                    last = dq == NPASS - 1
                    for m in [KIDX, 0, VIDX, 1, 2, 3]:
                        for tp in range(NT // 2):   # tt pairs share a psum
                            ps = psA.tile([P, 2 * QTW], F32, tag="pa")
                            for c in range(c_n):
                                for half in range(2):
                                    tt = tp * 2 + half
                                    nc.tensor.matmul(
                                        ps[:, half * QTW:(half + 1) * QTW],
                                        lhsT=wql[:, c, m * P:(m + 1) * P],
                                        rhs=xq[:, c,
                                               tt * QTW:(tt + 1) * QTW],
                                        start=(c == 0), stop=(c == c_n - 1))
                            accsl = acc[:, m,
                                        tp * 2 * QTW:(tp + 1) * 2 * QTW]
                            if dq == 0:
                                nc.scalar.copy(accsl, ps[:])
                            else:
                                nc.vector.tensor_add(
                                    accsl, accsl.bitcast(F32), ps[:])
                        if last:
                            if m == VIDX:
                                for c in range(NKT):  # v -> natural layout
                                    tps = psVT.tile([P, 2, P], F32R,
                                                    tag="vt")
                                    nc.tensor.transpose(
                                        tps[:, c % 2, :],
                                        acc[:, m, c * P:(c + 1) * P],
                                        ident_s[:])
                                    if c % 2 == 0:
                                        nc.scalar.copy(v_nat[:, c, :],
                                                       tps[:, 0, :])
                                    else:
                                        nc.vector.tensor_copy(
                                            v_nat[:, c, :], tps[:, 1, :])
                            else:
                                rope_m(m, psRot)

            # ---- fused B+C: attention + output projection ----
            with ExitStack() as bctx:
                psS = bctx.enter_context(
                    tc.tile_pool(name="psS", bufs=4, space="PSUM"))
                psO = bctx.enter_context(
                    tc.tile_pool(name="psO", bufs=2, space="PSUM"))
                psD = bctx.enter_context(
                    tc.tile_pool(name="psD", bufs=2, space="PSUM"))

                def emit_C(att, qt, ttl, b=b):
                    tt0 = qt * TPQ + ttl
                    for ho in range(2):
                        ob = obp.tile([P, D // 2], BF16, tag="ob")
                        for hf in range(D // QTW // 2):
                            half = ho * (D // QTW // 2) + hf
                            ps = psS.tile([P, QTW], F32, tag="s")
                            for hh in range(REP):
                                nc.tensor.matmul(
                                    ps[:],
                                    lhsT=att[hh][:, ttl * P:(ttl + 1) * P],
                                    rhs=wo_s[:, hh,
                                             half * QTW:(half + 1) * QTW],
                                    start=(hh == 0), stop=(hh == REP - 1))
                            if half % 2 == 0:
                                nc.scalar.copy(
                                    ob[:, hf * QTW:(hf + 1) * QTW], ps[:])
                            else:
                                nc.vector.tensor_copy(
                                    ob[:, hf * QTW:(hf + 1) * QTW], ps[:])
                        nc.scalar.dma_start(
                            out_d[b * S + tt0 * P:b * S + (tt0 + 1) * P,
                                  ho * (D // 2):(ho + 1) * (D // 2)],
                            ob[:])

                def pop_one():
                    en = pend.pop(0)
                    slot = en["slot"]
                    if slot["po"] is None:
                        slot["po"] = psO.tile([P, QTW], F32, tag="o", name="ps_o")
                        slot["pd"] = psD.tile([P, QTW], F32, tag="d", name="ps_d")
                    eo = en["eo"]
                    first = en["kt"] == 0
                    final = en["kt"] == en["nkt"] - 1
                    nc.tensor.matmul(
                        slot["po"][:, eo:], lhsT=v_nat[:, en["kt"], :],
                        rhs=en["e"][:, eo:], start=first, stop=final,
                        skip_group_check=True)
                    nc.tensor.matmul(
                        slot["pd"][:, eo:], lhsT=ones_s[:],
                        rhs=en["e"][:, eo:], start=first, stop=final,
                        skip_group_check=True)
                    if final:
                        rec = rp.tile([P, QTW], F32, tag="rec")
                        nc.vector.reciprocal(rec[:], slot["pd"][:])
                        at = attp.tile([P, QTW], BF16, tag=f"at{en['h']}")
                        nc.vector.tensor_mul(at[:], slot["po"][:], rec[:])
                        en["att"][en["h"]] = at
                        if en["h"] == REP - 1:
                            for ttl in range(TPQ):
                                cqueue.append((en["att"], en["qt"], ttl))

                for qi, qt in enumerate(QT_ORDER):
                    cur_att = att_first if qi == 0 else {}
                    for h in range(REP):
                        if cqueue:
                            emit_C(*cqueue.pop(0))
                        if len(cqueue) > 3:
                            emit_C(*cqueue.pop(0))
                        for _ in emit_scores(
                                qt, h, cur_att,
                                lambda: psS.tile([P, QTW], F32, tag="s",
                                                 name="ps_sc")):
                            while len(pend) > PIPE:
                                pop_one()
                        if h == REP - 1:
                            # drain harder so this qt's C work unlocks
                            # before the next (possibly tiny) blocks start
                            while len(pend) > PIPE // 2:
                                pop_one()
                while pend:
                    pop_one()
                while cqueue:
                    emit_C(*cqueue.pop(0))
    nc.compile()
    return nc


def get_nc():
    global _nc
    if _nc is None:
        _nc = _build_nc()
    return _nc


def make_in_maps(x, freqs_cos, freqs_sin, wq, wk, wv, wo):
    """Host-side prep: transposes, rope tables, masks, per-core weight shards."""
    import concourse.mybir as mybir
    BF16 = mybir.dt.np(mybir.dt.bfloat16)
    x = np.ascontiguousarray(x, np.float32)
    fc = np.asarray(freqs_cos, np.float32)
    fs = np.asarray(freqs_sin, np.float32)
    wq = np.asarray(wq, np.float32)
    wk = np.asarray(wk, np.float32)
    wv = np.asarray(wv, np.float32)
    wo = np.asarray(wo, np.float32)

    xT = np.ascontiguousarray(x.reshape(T, D).astype(BF16).T)
    cdup = np.ascontiguousarray(np.tile(np.repeat(fc.T, 2, axis=0), (1, B)))
    sdup = np.ascontiguousarray(np.tile(np.repeat(fs.T, 2, axis=0), (1, B)))
    prot = np.zeros((P, P), np.float32)
    for i in range(P // 2):
        prot[2 * i, 2 * i + 1] = -1.0
        prot[2 * i + 1, 2 * i] = 1.0
    pt = np.ascontiguousarray(prot.T)
    ones = np.ones((P, P), BF16)
    ident = np.eye(P, dtype=np.float32)
    ki = np.arange(KTW)[:, None]
    ci = np.arange(KTW)[None, :]
    tri = np.where(ki > ci, -1e9, 0.0).astype(np.float32)
    mtri = np.concatenate([np.full((P, KTW), -1e9, np.float32), tri], axis=1)
    mtri = np.ascontiguousarray(mtri)

    in_maps = []
    for g in range(NCORES):
        wq_g = wq[g * REP * HD:(g + 1) * REP * HD]
        wk_g = wk[g * HD:(g + 1) * HD]
        wv_g = wv[g * HD:(g + 1) * HD]
        wqkvT = np.ascontiguousarray(
            np.concatenate([wq_g, wk_g, wv_g], 0).astype(BF16).T)
        woT = np.ascontiguousarray(
            wo[:, g * REP * HD:(g + 1) * REP * HD].T.astype(BF16))
        in_maps.append({
            "xT": xT, "wqkvT": wqkvT, "woT": woT,
            "cdup": cdup, "sdup": sdup, "pt": pt, "ones": ones,
            "ident": ident, "mtri": mtri,
        })
    return in_maps


def kernel(x, freqs_cos, freqs_sin, wq, wk, wv, wo):
    from concourse.bass_utils import run_bass_kernel_spmd
    nc = get_nc()
    in_maps = make_in_maps(x, freqs_cos, freqs_sin, wq, wk, wv, wo)
    res = run_bass_kernel_spmd(nc, in_maps, core_ids=list(range(NCORES)))
    out = np.zeros((T, D), np.float64)
    for r in res.results:
        out += r["out"].astype(np.float64)
    return out.astype(np.float32).reshape(B, S, D)
